# revision 6
# baseline (speedup 1.0000x reference)
"""Trainium2 Bass kernel for nn_Attender_20263655702790.

Computation (reference):
  recal  = softmax(local_landmarks, axis=K)          # [N,B,K,H,W], K=13
  pooled = einsum("nbkhw,nbchw->bkc", recal, gmaps)  # [B,K,C]
  out    = pooled / max(||pooled||_2 over C, 1e-12)  # [B,K,C]

Shapes: N=2, B=32, K=13, C=512, H=W=28 (HW=784). All float32.

Sharding: data-parallel over B across 8 NeuronCores (4 batches per core).
Per-core DRAM inputs (prepared host-side in kernel()):
  lmk  [104, 784]        natural-layout landmarks, row = (n, b_local, k)
  gmap [2, 4, 784, 512]  global maps pre-transposed host-side to hw-major
Per-core output:
  out  [4, 13, 512]

Device algorithm per core:
  - DMA lmk; PE-transpose each [104, 112] hw-slab -> [112, 104] (hw on
    partitions, (n,b,k) on free).
  - Softmax over K without max-subtraction (inputs are unit normals, exp is
    safely in range): exp on ACT, 13-wide block sums on DVE, reciprocal,
    broadcast multiply.
  - For each local b: stream both n-slabs of gmap ([112, 7, 512] tiles),
    accumulate pooled[13, 512] in PSUM over 14 matmuls
    (lhsT = recal [112hw, 13k], rhs = gmap [112hw, 512c]).
  - L2 normalize over C on ACT/DVE, single output DMA.

The kernel is memory-bound: ~13.3 MB HBM traffic per core.
"""

import json

import numpy as np

import concourse.bass as bass
import concourse.tile as tile
from concourse import mybir
from concourse.masks import make_identity

F32 = mybir.dt.float32
AF = mybir.ActivationFunctionType

N = 2
B = 32
K = 13
C = 512
HW = 784
N_CORES = 8
BL = B // N_CORES  # 4 local batches per core
NB = N * BL  # 8 (n, b_local) pairs
P = 112  # hw tile partition count; 784 = 7 * 112
T = HW // P  # 7 hw tiles
EPS = 1e-12

_PATCHED = False


def _split_sync_waits(bir_bytes):
    """The pinned walrus build rejects instructions carrying more than one
    sem-wait (setupSyncWait: "Too many sync wait commands"). Tile sometimes
    attaches several waits to one instruction (matmuls with multiple
    producers, the kernel-tail drain). Hoist the extra waits onto injected
    same-engine NoOps immediately before the instruction — identical
    semantics, one wait per instruction."""
    m = json.loads(bir_bytes)
    counter = 0
    for fn in m.get("functions", []):
        for blk in fn.get("blocks", []):
            insts = blk.get("instructions")
            if not insts:
                continue
            out = []
            for inst in insts:
                si = inst.get("sync_info")
                if si:
                    waits = si.get("on_wait") or []
                    if len(waits) > 1:
                        for w in waits[:-1]:
                            counter += 1
                            out.append(
                                {
                                    "debug": inst.get("debug", 0),
                                    "engine": inst["engine"],
                                    "ins": [],
                                    "outs": [],
                                    "name": f"{inst['name']}-sw{counter}",
                                    "opcode": "NoOp",
                                    "sync_info": {
                                        "on_wait": [w],
                                        "on_update": [],
                                    },
                                }
                            )
                        si["on_wait"] = [waits[-1]]
                out.append(inst)
            blk["instructions"] = out
    return json.dumps(m).encode()


def _patch_compile():
    """Route every BIR compile through _split_sync_waits."""
    global _PATCHED
    if _PATCHED:
        return
    _PATCHED = True
    import concourse.bass_utils as bu
    import concourse.bass2jax as b2j

    orig = bu.compile_bir_kernel

    def patched(bir_json, tmpdir, neff_name="file.neff"):
        return orig(_split_sync_waits(bir_json), tmpdir, neff_name)

    bu.compile_bir_kernel = patched
    b2j.compile_bir_kernel = patched


def build_bass(mm_dtype="fp32"):
    """Build the per-core Bass module. mm_dtype: fp32 | fp32r."""
    _patch_compile()
    nc = bass.Bass()
    lmk = nc.dram_tensor("lmk", [NB * K, HW], F32, kind="ExternalInput")
    # gmap host layout: hw split as (t p), stored [n, b, p, t, c] so each SBUF
    # partition row is one contiguous 14 KB run (T*C floats) per (n, b) DMA.
    gmap = nc.dram_tensor("gmap", [N, BL, P, T, C], F32, kind="ExternalInput")
    out_d = nc.dram_tensor("out", [BL, K, C], F32, kind="ExternalOutput")

    def mm_ap(ap):
        if mm_dtype == "fp32r":
            return ap.bitcast(mybir.dt.float32r)
        return ap

    with tile.TileContext(nc) as tc:
        with (
            tc.tile_pool(name="consts", bufs=1) as consts,
            tc.tile_pool(name="lmp", bufs=1) as lmp,
            tc.tile_pool(name="recp", bufs=1) as recp,
            tc.tile_pool(name="gp", bufs=4) as gp,
            tc.tile_pool(name="small", bufs=8) as small,
            tc.tile_pool(name="outp", bufs=1) as outp,
            tc.tile_pool(name="ptp", bufs=2, space="PSUM") as ptp,
            tc.tile_pool(name="pmp", bufs=4, space="PSUM") as pmp,
        ):
            ident = consts.tile([NB * K, NB * K], F32)
            make_identity(nc, ident)
            sb_eps = consts.tile([K, 1], F32)
            nc.vector.memset(sb_eps, EPS)

            sb_lmk = lmp.tile([NB * K, HW], F32)
            nc.sync.dma_start(out=sb_lmk, in_=lmk[:, :])

            # recal[p_hw, t, (n b k)] after softmax over k
            sb_rec = recp.tile([P, T, NB * K], F32)
            sb_rcp = recp.tile([P, T, NB], F32)

            for t in range(T):
                pt = ptp.tile([P, NB * K], F32, tag="pt")
                nc.tensor.transpose(pt, sb_lmk[:, t * P : (t + 1) * P], ident)
                rec_t = sb_rec[:, t, :]
                nc.scalar.activation(rec_t, pt, AF.Exp)
                rec3 = rec_t.rearrange("p (nb k) -> p nb k", k=K)
                ssum = small.tile([P, NB], F32, tag="ssum")
                nc.vector.reduce_sum(ssum, rec3, axis=mybir.AxisListType.X)
                nc.vector.reciprocal(sb_rcp[:, t, :], ssum)
                nc.vector.tensor_mul(
                    rec3,
                    rec3,
                    sb_rcp[:, t, :].unsqueeze(2).broadcast_to([P, NB, K]),
                )

            sb_out = outp.tile([K, BL, C], F32)
            for b in range(BL):
                g_tiles = []
                for n in range(N):
                    sb_g = gp.tile([P, T, C], F32, tag="g")
                    nc.sync.dma_start(out=sb_g, in_=gmap[n, b])
                    g_tiles.append(sb_g)
                pm = pmp.tile([K, C], F32, tag="pm")
                for n in range(N):
                    nb = n * BL + b
                    for t in range(T):
                        nc.tensor.matmul(
                            pm,
                            mm_ap(sb_rec[:, t, nb * K : (nb + 1) * K]),
                            mm_ap(g_tiles[n][:, t, :]),
                            start=(n == 0 and t == 0),
                            stop=(n == N - 1 and t == T - 1),
                        )
                # L2 normalize over C
                sq = small.tile([K, C], F32, tag="sq")
                ss = small.tile([K, 1], F32, tag="ss")
                nc.scalar.activation(sq, pm, AF.Square, accum_out=ss)
                nrm = small.tile([K, 1], F32, tag="nrm")
                nc.scalar.activation(nrm, ss, AF.Sqrt)
                nc.vector.tensor_max(nrm, nrm, sb_eps)
                rcpn = small.tile([K, 1], F32, tag="rcpn")
                nc.vector.reciprocal(rcpn, nrm)
                nc.vector.tensor_scalar_mul(sb_out[:, b, :], in0=pm, scalar1=rcpn)

            nc.sync.dma_start(out=out_d.rearrange("b k c -> k b c"), in_=sb_out)
    return nc


def _prep_in_maps(local_landmarks, global_maps):
    l = np.ascontiguousarray(np.asarray(local_landmarks, dtype=np.float32)).reshape(
        N, B, K, HW
    )
    g = np.asarray(global_maps, dtype=np.float32).reshape(N, B, C, T, P)
    # [N, B, P, T, C]: hw = t*P + p; partition rows contiguous per (n, b).
    gt = g.transpose(0, 1, 4, 3, 2)
    in_maps = []
    for c in range(N_CORES):
        bs = slice(BL * c, BL * (c + 1))
        in_maps.append(
            {
                "lmk": np.ascontiguousarray(l[:, bs]).reshape(NB * K, HW),
                "gmap": np.ascontiguousarray(gt[:, bs]),
            }
        )
    return in_maps


def run_on_cores(local_landmarks, global_maps, trace=False, mm_dtype="fp32"):
    """Returns (full_output [32,13,512], BassKernelResults)."""
    from concourse.bass_utils import run_bass_kernel_spmd

    nc = build_bass(mm_dtype=mm_dtype)
    in_maps = _prep_in_maps(local_landmarks, global_maps)
    res = run_bass_kernel_spmd(
        nc, in_maps, core_ids=list(range(N_CORES)), trace=trace
    )
    out = np.concatenate([r["out"] for r in res.results], axis=0)
    return np.ascontiguousarray(out, dtype=np.float32), res


def kernel(local_landmarks, global_maps):
    out, _ = run_on_cores(local_landmarks, global_maps, trace=False)
    return out


# revision 8
# speedup vs baseline: 1.3973x; 1.3973x over previous
"""Trainium2 Bass kernel for nn_Attender_20263655702790.

Computation (reference):
  recal  = softmax(local_landmarks, axis=K)          # [N,B,K,H,W], K=13
  pooled = einsum("nbkhw,nbchw->bkc", recal, gmaps)  # [B,K,C]
  out    = pooled / max(||pooled||_2 over C, 1e-12)  # [B,K,C]

Shapes: N=2, B=32, K=13, C=512, H=W=28 (HW=784). All float32.

Sharding: data-parallel over B across 8 NeuronCores (4 batches per core).
Per-core DRAM inputs (prepared host-side in kernel()):
  lmk  [104, 784]        natural-layout landmarks, row = (n, b_local, k)
  gmap [2, 4, 784, 512]  global maps pre-transposed host-side to hw-major
Per-core output:
  out  [4, 13, 512]

Device algorithm per core:
  - DMA lmk; PE-transpose each [104, 112] hw-slab -> [112, 104] (hw on
    partitions, (n,b,k) on free).
  - Softmax over K without max-subtraction (inputs are unit normals, exp is
    safely in range): exp on ACT, 13-wide block sums on DVE, reciprocal,
    broadcast multiply.
  - For each local b: stream both n-slabs of gmap ([112, 7, 512] tiles),
    accumulate pooled[13, 512] in PSUM over 14 matmuls
    (lhsT = recal [112hw, 13k], rhs = gmap [112hw, 512c]).
  - L2 normalize over C on ACT/DVE, single output DMA.

The kernel is memory-bound: ~13.3 MB HBM traffic per core.
"""

import json

import numpy as np

import concourse.bass as bass
import concourse.tile as tile
from concourse import mybir
from concourse.masks import make_identity

F32 = mybir.dt.float32
AF = mybir.ActivationFunctionType

N = 2
B = 32
K = 13
C = 512
HW = 784
N_CORES = 8
BL = B // N_CORES  # 4 local batches per core
NB = N * BL  # 8 (n, b_local) pairs
P = 112  # hw tile partition count; 784 = 7 * 112
T = HW // P  # 7 hw tiles
EPS = 1e-12

_PATCHED = False


def _split_sync_waits(bir_bytes):
    """The pinned walrus build rejects instructions carrying more than one
    sem-wait (setupSyncWait: "Too many sync wait commands"). Tile sometimes
    attaches several waits to one instruction (matmuls with multiple
    producers, the kernel-tail drain). Hoist the extra waits onto injected
    same-engine NoOps immediately before the instruction — identical
    semantics, one wait per instruction."""
    m = json.loads(bir_bytes)
    counter = 0
    for fn in m.get("functions", []):
        for blk in fn.get("blocks", []):
            insts = blk.get("instructions")
            if not insts:
                continue
            out = []
            for inst in insts:
                si = inst.get("sync_info")
                if si:
                    waits = si.get("on_wait") or []
                    if len(waits) > 1:
                        for w in waits[:-1]:
                            counter += 1
                            out.append(
                                {
                                    "debug": inst.get("debug", 0),
                                    "engine": inst["engine"],
                                    "ins": [],
                                    "outs": [],
                                    "name": f"{inst['name']}-sw{counter}",
                                    "opcode": "NoOp",
                                    "sync_info": {
                                        "on_wait": [w],
                                        "on_update": [],
                                    },
                                }
                            )
                        si["on_wait"] = [waits[-1]]
                out.append(inst)
            blk["instructions"] = out
    return json.dumps(m).encode()


def _patch_compile():
    """Route every BIR compile through _split_sync_waits."""
    global _PATCHED
    if _PATCHED:
        return
    _PATCHED = True
    import concourse.bass_utils as bu
    import concourse.bass2jax as b2j

    orig = bu.compile_bir_kernel

    def patched(bir_json, tmpdir, neff_name="file.neff"):
        return orig(_split_sync_waits(bir_json), tmpdir, neff_name)

    bu.compile_bir_kernel = patched
    b2j.compile_bir_kernel = patched


def build_bass(mm_dtype="fp32"):
    """Build the per-core Bass module. mm_dtype: fp32 | fp32r."""
    _patch_compile()
    nc = bass.Bass()
    # float32r is bit-identical to f32 in memory; typing the operand chain as
    # f32r lets the PE run matmuls at 1 cycle/row instead of fp32's 4.
    MD = mybir.dt.float32r if mm_dtype == "fp32r" else F32
    lmk = nc.dram_tensor("lmk", [NB * K, HW], F32, kind="ExternalInput")
    # gmap host layout: hw split as (t p), stored [n, b, p, t, c] so each SBUF
    # partition row is one contiguous 14 KB run (T*C floats) per (n, b) DMA.
    gmap = nc.dram_tensor("gmap", [N, BL, P, T, C], MD, kind="ExternalInput")
    out_d = nc.dram_tensor("out", [BL, K, C], F32, kind="ExternalOutput")

    with tile.TileContext(nc) as tc:
        with (
            tc.tile_pool(name="consts", bufs=1) as consts,
            tc.tile_pool(name="lmp", bufs=1) as lmp,
            tc.tile_pool(name="recp", bufs=1) as recp,
            tc.tile_pool(name="gp", bufs=4) as gp,
            tc.tile_pool(name="small", bufs=8) as small,
            tc.tile_pool(name="outp", bufs=1) as outp,
            tc.tile_pool(name="ptp", bufs=2, space="PSUM") as ptp,
            tc.tile_pool(name="pmp", bufs=4, space="PSUM") as pmp,
        ):
            ident = consts.tile([NB * K, NB * K], F32)
            make_identity(nc, ident)
            sb_eps = consts.tile([K, 1], F32)
            nc.vector.memset(sb_eps, EPS)

            sb_lmk = lmp.tile([NB * K, HW], F32)
            nc.sync.dma_start(out=sb_lmk, in_=lmk[:, :])

            # recal[p_hw, t, (n b k)] after softmax over k
            sb_rec = recp.tile([P, T, NB * K], MD)
            sb_rcp = recp.tile([P, T, NB], F32)

            for t in range(T):
                pt = ptp.tile([P, NB * K], F32, tag="pt")
                nc.tensor.transpose(pt, sb_lmk[:, t * P : (t + 1) * P], ident)
                rec_t = sb_rec[:, t, :]
                nc.scalar.activation(rec_t, pt, AF.Exp)
                rec3 = rec_t.rearrange("p (nb k) -> p nb k", k=K)
                ssum = small.tile([P, NB], F32, tag="ssum")
                nc.vector.reduce_sum(ssum, rec3, axis=mybir.AxisListType.X)
                nc.vector.reciprocal(sb_rcp[:, t, :], ssum)
                nc.vector.tensor_mul(
                    rec3,
                    rec3,
                    sb_rcp[:, t, :].unsqueeze(2).broadcast_to([P, NB, K]),
                )

            sb_out = outp.tile([K, BL, C], F32)
            for b in range(BL):
                g_tiles = []
                for n in range(N):
                    sb_g = gp.tile([P, T, C], MD, tag="g")
                    nc.sync.dma_start(out=sb_g, in_=gmap[n, b])
                    g_tiles.append(sb_g)
                pm = pmp.tile([K, C], F32, tag="pm")
                for n in range(N):
                    nb = n * BL + b
                    for t in range(T):
                        nc.tensor.matmul(
                            pm,
                            sb_rec[:, t, nb * K : (nb + 1) * K],
                            g_tiles[n][:, t, :],
                            start=(n == 0 and t == 0),
                            stop=(n == N - 1 and t == T - 1),
                        )
                # L2 normalize over C
                sq = small.tile([K, C], F32, tag="sq")
                ss = small.tile([K, 1], F32, tag="ss")
                nc.scalar.activation(sq, pm, AF.Square, accum_out=ss)
                nrm = small.tile([K, 1], F32, tag="nrm")
                nc.scalar.activation(nrm, ss, AF.Sqrt)
                nc.vector.tensor_max(nrm, nrm, sb_eps)
                rcpn = small.tile([K, 1], F32, tag="rcpn")
                nc.vector.reciprocal(rcpn, nrm)
                nc.vector.tensor_scalar_mul(sb_out[:, b, :], in0=pm, scalar1=rcpn)

            nc.sync.dma_start(out=out_d.rearrange("b k c -> k b c"), in_=sb_out)
    return nc


def _prep_in_maps(local_landmarks, global_maps):
    l = np.ascontiguousarray(np.asarray(local_landmarks, dtype=np.float32)).reshape(
        N, B, K, HW
    )
    g = np.asarray(global_maps, dtype=np.float32).reshape(N, B, C, T, P)
    # [N, B, P, T, C]: hw = t*P + p; partition rows contiguous per (n, b).
    gt = g.transpose(0, 1, 4, 3, 2)
    in_maps = []
    for c in range(N_CORES):
        bs = slice(BL * c, BL * (c + 1))
        in_maps.append(
            {
                "lmk": np.ascontiguousarray(l[:, bs]).reshape(NB * K, HW),
                "gmap": np.ascontiguousarray(gt[:, bs]),
            }
        )
    return in_maps


def run_on_cores(local_landmarks, global_maps, trace=False, mm_dtype="fp32"):
    """Returns (full_output [32,13,512], BassKernelResults)."""
    from concourse.bass_utils import run_bass_kernel_spmd

    nc = build_bass(mm_dtype=mm_dtype)
    in_maps = _prep_in_maps(local_landmarks, global_maps)
    res = run_bass_kernel_spmd(
        nc, in_maps, core_ids=list(range(N_CORES)), trace=trace
    )
    out = np.concatenate([r["out"] for r in res.results], axis=0)
    return np.ascontiguousarray(out, dtype=np.float32), res


def kernel(local_landmarks, global_maps):
    out, _ = run_on_cores(local_landmarks, global_maps, trace=False)
    return out
